# revision 21
# baseline (speedup 1.0000x reference)
"""Distributed CLIP-style focal contrastive loss on 8 Trainium2 NeuronCores.

Math (reference): normalize img/txt rows, logits = img_n @ txt_n.T / T,
row/col log-softmax diagonals, focal-weighted mean.

Sharding: each core owns a 1024-row TEXT shard. Per core the logits block is
computed transposed: psum[p=txt_j, n=img_i] = txt_raw_j . (img_i / |img_i|),
with the txt norm and 1/T folded into the exp's per-partition scale, and a
constant bias of -1/T making exp(arg) <= 1 (stable softmax without a max
pass). The ScalarE exp's accum_out gives the complete t2i denominators for
the shard; a ones-matmul on PE (partition reduction) accumulated over row
blocks in PSUM gives partial i2t denominators, reduced across cores on the
host. Diagonal logits are computed separately from the shard vectors.
The final O(B) log/focal/mean is done on the host in float64.

Matmul operands are cast to bf16 (single-pass PE; fp32 matmuls lower to two
HW passes). The resulting per-logit noise (~6e-3 absolute) is zero-mean and
averages out over the 8192-term loss mean (~1e-4 relative on the loss).
Norms use exp(-0.5*ln(ss)) on ScalarE: same ACT table set as the main exp,
and avoids DVE reciprocal (iterative divide, ~5 ns/elem).
"""

import math
import os
import sys
import types

sys.path.insert(0, "/opt/trn_rl_repo")

import numpy as np

import concourse.bacc as bacc
import concourse.bass as bass
import concourse.tile as tile
from concourse import mybir
from concourse.bass_utils import run_bass_kernel_spmd

B = 8192
D = 64
NCORES = 8
S = B // NCORES  # 1024 txt rows per core
RB = S // 128  # 8 row blocks per shard
NG = 8  # img column groups
GW = B // NG  # 1024 img columns per group
TEMP = 0.07
CMAX = 1.0 / TEMP  # upper bound on any logit; used as the exp bias
F32 = mybir.dt.float32
BF16 = mybir.dt.bfloat16
AX = mybir.AxisListType
AF = mybir.ActivationFunctionType

LAST_EXEC_TIME_NS = None


def _maybe_enable_trace():
    """Wire up the NTFF profile hook (missing from this image's antenv) so
    run_bass_kernel_spmd(trace=True) can fetch real HW profiles via axon."""
    if not os.environ.get("BASS_KERNEL_TRACE"):
        return False
    try:
        if "antenv.axon_hooks" not in sys.modules:
            import trn_agent_boot.trn_boot as tb

            hook = tb._ntff_profile_via_ctypes("/opt/axon/libaxon_pjrt.so")
            mod = types.ModuleType("antenv.axon_hooks")
            mod.get_axon_ntff_profile_hook = lambda: hook
            sys.modules["antenv.axon_hooks"] = mod
        import concourse.bass_utils as bu

        bu.upload_artifacts = lambda tmpdir: "local://" + tmpdir
        return True
    except Exception:
        return False


def build_nc():
    nc = bacc.Bacc(None, target_bir_lowering=False)

    imgT = nc.dram_tensor("imgT", [D, B], F32, kind="ExternalInput")
    imgTs = nc.dram_tensor("imgTs", [D, S], F32, kind="ExternalInput")
    txtTs = nc.dram_tensor("txtTs", [D, S], F32, kind="ExternalInput")
    txtn = nc.dram_tensor("txtn", [S, D], F32, kind="ExternalInput")

    rowp = nc.dram_tensor("rowp", [1, B], F32, kind="ExternalOutput")
    colsum = nc.dram_tensor("colsum", [128, RB], F32, kind="ExternalOutput")
    diag = nc.dram_tensor("diag", [1, S], F32, kind="ExternalOutput")

    ss_dram = nc.dram_tensor("ss_scratch", [B], F32, kind="Internal")
    s_dram = nc.dram_tensor("s_scratch", [B], F32, kind="Internal")
    b_dram = nc.dram_tensor("b_scratch", [S], F32, kind="Internal")

    with tile.TileContext(nc) as tc:
        with (
            tc.tile_pool(name="big", bufs=1) as big,
            tc.tile_pool(name="chunks", bufs=2) as chunks,
            tc.tile_pool(name="small", bufs=1) as small,
            tc.tile_pool(name="etile", bufs=RB + 2) as etile,
            tc.tile_pool(name="acc", bufs=4) as accp,
            tc.tile_pool(name="tmp", bufs=2) as tmpp,
            tc.tile_pool(name="psum_mm", bufs=3, space="PSUM") as psum,
            tc.tile_pool(name="psum_vec", bufs=1, space="PSUM") as psumv,
        ):
            # ---------------- constants / persistent tiles ----------------
            ones64 = small.tile([D, 1], F32)
            nc.vector.memset(ones64, 1.0)
            ones128 = small.tile([128, 1], BF16)
            nc.vector.memset(ones128, 1.0)
            negC = small.tile([128, 1], F32)
            nc.vector.memset(negC, -CMAX)
            lninvT = small.tile([128, 1], F32)
            nc.vector.memset(lninvT, math.log(1.0 / TEMP))
            colsum_sb = small.tile([128, RB], F32)
            nc.vector.memset(colsum_sb, 0.0)

            imgT_sb = big.tile([D, B], F32)
            s_bc = big.tile([D, B], F32)
            txtTs_sb = small.tile([D, S], F32)
            txtTs_bf = small.tile([D, S], BF16)
            txtn_sb = small.tile([128, RB, D], F32)
            imgTs_sb = small.tile([D, S], F32)

            nc.sync.dma_start(txtTs_sb, txtTs[:, :])
            nc.sync.dma_start(imgTs_sb, imgTs[:, :])
            nc.sync.dma_start(
                txtn_sb, txtn[:, :].rearrange("(t p) d -> p t d", p=128)
            )
            nc.vector.tensor_copy(txtTs_bf, txtTs_sb)

            # ---------------- txt-side scales b = 1/(T*|y_j|) -------------
            # natural layout [p, t]: j = t*128 + p; b = exp(-ln(ss)/2 + ln(1/T))
            sq_t = small.tile([128, RB, D], F32)
            nc.vector.tensor_mul(sq_t, txtn_sb, txtn_sb)
            b_nat = small.tile([128, RB], F32)
            nc.vector.reduce_sum(b_nat, sq_t, axis=AX.X)
            nc.scalar.activation(b_nat, b_nat, AF.Ln)
            nc.scalar.activation(b_nat, b_nat, AF.Exp, bias=lninvT, scale=-0.5)

            # b in row layout [1, S] for the diagonal (j = 128*t + p),
            # via a DRAM bounce (partition<->free transposes don't balance
            # as direct SBUF->SBUF DMAs)
            b_row = small.tile([1, S], F32)
            nc.sync.dma_start(
                b_dram[:].rearrange("(t p) -> p t", p=128), b_nat[:, :]
            )
            nc.sync.dma_start(b_row, b_dram[:].rearrange("(a n) -> a n", a=1))

            # ---------------- img-side scales s_i = 1/|x_i| ---------------
            # Pipelined in NG rounds so the main loop can start after round 0.
            s_nat = small.tile([128, B // 128], F32)
            for r in range(NG):
                csl = slice(r * GW, (r + 1) * GW)
                nsl = slice(r * (GW // 128), (r + 1) * (GW // 128))
                nc.sync.dma_start(imgT_sb[:, csl], imgT[:, csl])
                sq_r = chunks.tile([D, GW], F32, tag="sq")
                nc.vector.tensor_mul(sq_r, imgT_sb[:, csl], imgT_sb[:, csl])
                ss_ps = psumv.tile([1, GW], F32, tag="vec")
                for k in range(GW // 512):
                    nc.tensor.matmul(
                        ss_ps[0:1, k * 512 : (k + 1) * 512],
                        ones64,
                        sq_r[:, k * 512 : (k + 1) * 512],
                        start=True,
                        stop=True,
                    )
                ss_row = chunks.tile([1, GW], F32, tag="ssrow")
                nc.vector.tensor_copy(ss_row, ss_ps)
                nc.sync.dma_start(
                    ss_dram[csl].rearrange("(a n) -> a n", a=1), ss_row
                )
                nc.sync.dma_start(
                    s_nat[:, nsl],
                    ss_dram[csl].rearrange("(t p) -> p t", p=128),
                )
                nc.scalar.activation(s_nat[:, nsl], s_nat[:, nsl], AF.Ln)
                nc.scalar.activation(
                    s_nat[:, nsl], s_nat[:, nsl], AF.Exp, scale=-0.5
                )
                nc.sync.dma_start(
                    s_dram[csl].rearrange("(t p) -> p t", p=128),
                    s_nat[:, nsl],
                )
                sd = s_dram[:].rearrange("(a n) -> a n", a=1)[0:1, csl]
                nc.gpsimd.dma_start(
                    s_bc[:, csl],
                    bass.AP(
                        tensor=sd.tensor,
                        offset=sd.offset,
                        ap=[[0, D]] + list(sd.ap[1:]),
                    ),
                )

            # ---------------- diagonal logits ------------------------------
            # d_j = (x_j . y_j) * (1/|x_j|) * (1/(T |y_j|))
            dmul = chunks.tile([D, S], F32, tag="dmul")
            nc.vector.tensor_mul(dmul, imgTs_sb, txtTs_sb)
            dot_ps = psumv.tile([1, S], F32, tag="vec")
            for k in range(S // 512):
                nc.tensor.matmul(
                    dot_ps[0:1, k * 512 : (k + 1) * 512],
                    ones64,
                    dmul[:, k * 512 : (k + 1) * 512],
                    start=True,
                    stop=True,
                )
            dot_sb = small.tile([1, S], F32)
            nc.vector.tensor_copy(dot_sb, dot_ps)
            sqis = chunks.tile([D, S], F32, tag="dmul")
            nc.vector.tensor_mul(sqis, imgTs_sb, imgTs_sb)
            ssis_ps = psumv.tile([1, S], F32, tag="vec")
            for k in range(S // 512):
                nc.tensor.matmul(
                    ssis_ps[0:1, k * 512 : (k + 1) * 512],
                    ones64,
                    sqis[:, k * 512 : (k + 1) * 512],
                    start=True,
                    stop=True,
                )
            s_is = small.tile([1, S], F32)
            nc.scalar.activation(s_is, ssis_ps, AF.Ln)
            nc.scalar.activation(s_is, s_is, AF.Exp, scale=-0.5)
            d_sb = small.tile([1, S], F32)
            nc.vector.tensor_mul(d_sb, dot_sb, s_is)
            nc.vector.tensor_mul(d_sb, d_sb, b_row)
            nc.sync.dma_start(diag[0:1, :], d_sb)

            # ---------------- main loop ------------------------------------
            # Per (g, rb): bf16 matmuls -> psum, exp(scale*psum - C) -> E
            # (accum_out = t2i partials). The 8 E tiles of a group are summed
            # elementwise on DVE+Pool (pairwise tree, overlapped with the
            # matmul stream), so the PE partition-reduce for the i2t partials
            # is ONE ones-matmul pass per group instead of one per E tile —
            # that reduce used to cost as many PE columns as the matmuls
            # themselves.  The final add emits bf16 so the ones-matmul stays
            # single-pass; the rounding is zero-mean and vanishes in the mean.
            NK = GW // 512

            def ones_pass(rs, accb, k):
                nc.tensor.matmul(
                    rs[0:1, k * 512 : (k + 1) * 512],
                    ones128,
                    accb[:, k * 512 : (k + 1) * 512],
                    start=True,
                    stop=True,
                )

            def flush_rowp(rs, g_idx):
                rowp_sb = chunks.tile([1, GW], F32, tag="rowp")
                nc.vector.tensor_copy(rowp_sb, rs)
                nc.sync.dma_start(
                    rowp[0:1, g_idx * GW : (g_idx + 1) * GW], rowp_sb
                )

            prev_accb = None
            for g in range(NG):
                gsl = slice(g * GW, (g + 1) * GW)
                imgTn_g = chunks.tile([D, GW], BF16, tag="imgTn")
                nc.vector.tensor_mul(imgTn_g, imgT_sb[:, gsl], s_bc[:, gsl])
                cur_e = []
                for rb in range(RB):
                    psum_t = psum.tile([128, GW], F32, tag="mm")
                    lhsT = txtTs_bf[:, rb * 128 : (rb + 1) * 128]
                    for k in range(NK):
                        nc.tensor.matmul(
                            psum_t[:, k * 512 : (k + 1) * 512],
                            lhsT,
                            imgTn_g[:, k * 512 : (k + 1) * 512],
                            start=True,
                            stop=True,
                        )
                        # prev group's reduced tile is ready by now; its two
                        # ones-passes slot between main matmuls (alternating
                        # weight sets keeps LDWEIGHTS in the background).
                        if prev_accb is not None and rb == 4:
                            if k == 0:
                                rs_ps = psumv.tile([1, GW], F32, tag="vec")
                            ones_pass(rs_ps, prev_accb, k)
                    e_t = etile.tile([128, GW], BF16, tag="e")
                    acc = accp.tile([128, 1], F32, tag="acc")
                    nc.scalar.activation(
                        e_t,
                        psum_t,
                        AF.Exp,
                        bias=negC,
                        scale=b_nat[:, rb : rb + 1],
                        accum_out=acc,
                    )
                    nc.gpsimd.tensor_add(
                        colsum_sb[:, rb : rb + 1],
                        colsum_sb[:, rb : rb + 1],
                        acc,
                    )
                    cur_e.append(e_t)
                    if rb == 4 and prev_accb is not None:
                        flush_rowp(rs_ps, g - 1)
                    # pairwise E reduction, eager, split DVE/Pool
                    if rb == 1:
                        t01 = tmpp.tile([128, GW], F32, tag="t01")
                        nc.vector.tensor_add(t01, cur_e[0], cur_e[1])
                    elif rb == 3:
                        t23 = tmpp.tile([128, GW], F32, tag="t23")
                        nc.gpsimd.tensor_add(t23, cur_e[2], cur_e[3])
                        nc.vector.tensor_add(t01, t01, t23)
                    elif rb == 5:
                        t45 = tmpp.tile([128, GW], F32, tag="t45")
                        nc.vector.tensor_add(t45, cur_e[4], cur_e[5])
                    elif rb == 7:
                        t67 = tmpp.tile([128, GW], F32, tag="t67")
                        nc.gpsimd.tensor_add(t67, cur_e[6], cur_e[7])
                        nc.gpsimd.tensor_add(t45, t45, t67)
                        accb = tmpp.tile([128, GW], BF16, tag="accb")
                        nc.vector.tensor_add(accb, t01, t45)
                prev_accb = accb
            rs_ps = psumv.tile([1, GW], F32, tag="vec")
            for k in range(NK):
                ones_pass(rs_ps, prev_accb, k)
            flush_rowp(rs_ps, NG - 1)

            nc.sync.dma_start(colsum[:, :], colsum_sb)

    nc.compile()
    _patch_act_table_loads(nc)
    return nc


def _patch_act_table_loads(nc):
    """All ACT funcs used here (Ln, Exp) live in one table set
    ('natural_log_exp_and_others'), but the insertion pass picks a
    different set per function, reloading tables (~1.3us each) on every
    Ln<->Exp transition. Point every load at the combined set and drop
    the now-redundant reloads (keeping any that carry semaphore ops)."""
    from concourse.hw_specs import get_activation_tables

    tables = list(get_activation_tables(nc.m.arch).items())
    want = {"ln", "exp"}
    idx = next(
        i
        for i, (name, funcs) in enumerate(tables)
        if name == "natural_log_exp_and_others"
    )
    assert want <= {f.name.lower() for f in tables[idx][1]} or True
    seen = False
    for blk in nc.main_func.blocks:
        drop = []
        for ins in blk.instructions:
            if not isinstance(ins, mybir.InstLoadActFuncSet):
                continue
            ins.act_func_set_id = idx
            si = getattr(ins, "sync_info", None)
            clean = si is None or (not si.on_wait and not si.on_update)
            if seen and clean:
                drop.append(ins)
            seen = True
        for ins in drop:
            blk.instructions.remove(ins)


_NC = None


def _get_nc():
    global _NC
    if _NC is None:
        _NC = build_nc()
    return _NC


def kernel(image_features, text_features):
    img = np.ascontiguousarray(np.asarray(image_features, dtype=np.float32))
    txt = np.ascontiguousarray(np.asarray(text_features, dtype=np.float32))
    assert img.shape == (B, D) and txt.shape == (B, D)

    imgT = np.ascontiguousarray(img.T)
    in_maps = []
    for c in range(NCORES):
        sh = slice(c * S, (c + 1) * S)
        in_maps.append(
            {
                "imgT": imgT,
                "imgTs": np.ascontiguousarray(img[sh].T),
                "txtTs": np.ascontiguousarray(txt[sh].T),
                "txtn": np.ascontiguousarray(txt[sh]),
            }
        )

    trace = _maybe_enable_trace()
    res = run_bass_kernel_spmd(
        _get_nc(), in_maps, core_ids=list(range(NCORES)), trace=trace
    )
    global LAST_EXEC_TIME_NS
    LAST_EXEC_TIME_NS = res.exec_time_ns

    rowsum = np.zeros(B, dtype=np.float64)
    colsum = np.empty(B, dtype=np.float64)
    diag = np.empty(B, dtype=np.float64)
    for c in range(NCORES):
        r = res.results[c]
        rowsum += r["rowp"][0].astype(np.float64)
        colsum[c * S : (c + 1) * S] = r["colsum"].T.reshape(-1)
        diag[c * S : (c + 1) * S] = r["diag"][0]

    # exp was computed with bias -CMAX, so logZ = CMAX + log(sum)
    dlr = diag - (np.log(rowsum) + CMAX)  # diag of row log-softmax (i2t)
    dlc = diag - (np.log(colsum) + CMAX)  # diag of col log-softmax (t2i)
    p_pos = np.exp(dlr)
    w = (1.0 - p_pos) ** 2
    loss = 0.5 * (np.mean(w * -dlr) + np.mean(w * -dlc))
    return np.array(loss, dtype=np.float32)


# revision 24
# speedup vs baseline: 1.4348x; 1.4348x over previous
"""Distributed CLIP-style focal contrastive loss on 8 Trainium2 NeuronCores.

Math (reference): normalize img/txt rows, logits = img_n @ txt_n.T / T,
row/col log-softmax diagonals, focal-weighted mean.

Sharding: each core owns a 1024-row TEXT shard. Per core the logits block is
computed transposed: psum[p=txt_j, n=img_i] = txt_raw_j . (img_i / |img_i|),
with the txt norm and 1/T folded into the exp's per-partition scale, and a
constant bias of -1/T making exp(arg) <= 1 (stable softmax without a max
pass). The ScalarE exp's accum_out gives the complete t2i denominators for
the shard; a ones-matmul on PE (partition reduction) accumulated over row
blocks in PSUM gives partial i2t denominators, reduced across cores on the
host. Diagonal logits are computed separately from the shard vectors.
The final O(B) log/focal/mean is done on the host in float64.

Matmul operands are cast to bf16 (single-pass PE; fp32 matmuls lower to two
HW passes). The resulting per-logit noise (~6e-3 absolute) is zero-mean and
averages out over the 8192-term loss mean (~1e-4 relative on the loss).
Norms use exp(-0.5*ln(ss)) on ScalarE: same ACT table set as the main exp,
and avoids DVE reciprocal (iterative divide, ~5 ns/elem).
"""

import math
import os
import sys
import types

sys.path.insert(0, "/opt/trn_rl_repo")

import numpy as np

import concourse.bacc as bacc
import concourse.bass as bass
import concourse.tile as tile
from concourse import mybir
from concourse.bass_utils import run_bass_kernel_spmd

B = 8192
D = 64
NCORES = 8
S = B // NCORES  # 1024 txt rows per core
RB = S // 128  # 8 row blocks per shard
NG = 8  # img column groups
GW = B // NG  # 1024 img columns per group
TEMP = 0.07
CMAX = 1.0 / TEMP  # upper bound on any logit; used as the exp bias
F32 = mybir.dt.float32
BF16 = mybir.dt.bfloat16
AX = mybir.AxisListType
AF = mybir.ActivationFunctionType

LAST_EXEC_TIME_NS = None


def _maybe_enable_trace():
    """Wire up the NTFF profile hook (missing from this image's antenv) so
    run_bass_kernel_spmd(trace=True) can fetch real HW profiles via axon."""
    if not os.environ.get("BASS_KERNEL_TRACE"):
        return False
    try:
        if "antenv.axon_hooks" not in sys.modules:
            import trn_agent_boot.trn_boot as tb

            hook = tb._ntff_profile_via_ctypes("/opt/axon/libaxon_pjrt.so")
            mod = types.ModuleType("antenv.axon_hooks")
            mod.get_axon_ntff_profile_hook = lambda: hook
            sys.modules["antenv.axon_hooks"] = mod
        import concourse.bass_utils as bu

        bu.upload_artifacts = lambda tmpdir: "local://" + tmpdir
        return True
    except Exception:
        return False


def build_nc():
    nc = bacc.Bacc(None, target_bir_lowering=False)

    imgT = nc.dram_tensor("imgT", [D, B], F32, kind="ExternalInput")
    imgTs = nc.dram_tensor("imgTs", [D, S], F32, kind="ExternalInput")
    txtTs = nc.dram_tensor("txtTs", [D, S], F32, kind="ExternalInput")
    txtn = nc.dram_tensor("txtn", [S, D], F32, kind="ExternalInput")

    rowp = nc.dram_tensor("rowp", [1, B], F32, kind="ExternalOutput")
    colsum = nc.dram_tensor("colsum", [128, RB], F32, kind="ExternalOutput")
    diag = nc.dram_tensor("diag", [1, S], F32, kind="ExternalOutput")

    ss_dram = nc.dram_tensor("ss_scratch", [B], F32, kind="Internal")
    s_dram = nc.dram_tensor("s_scratch", [B], F32, kind="Internal")
    b_dram = nc.dram_tensor("b_scratch", [S], F32, kind="Internal")

    with tile.TileContext(nc) as tc:
        with (
            tc.tile_pool(name="big", bufs=1) as big,
            tc.tile_pool(name="chunks", bufs=2) as chunks,
            tc.tile_pool(name="small", bufs=1) as small,
            tc.tile_pool(name="etile", bufs=RB + 2) as etile,
            tc.tile_pool(name="acc", bufs=4) as accp,
            tc.tile_pool(name="tmp", bufs=2) as tmpp,
            tc.tile_pool(name="psum_mm", bufs=3, space="PSUM") as psum,
            tc.tile_pool(name="psum_vec", bufs=1, space="PSUM") as psumv,
        ):
            # ---------------- constants / persistent tiles ----------------
            ones64 = small.tile([D, 1], F32)
            nc.vector.memset(ones64, 1.0)
            ones128 = small.tile([128, 1], BF16)
            nc.vector.memset(ones128, 1.0)
            negC = small.tile([128, 1], F32)
            nc.vector.memset(negC, -CMAX)
            lninvT = small.tile([128, 1], F32)
            nc.vector.memset(lninvT, math.log(1.0 / TEMP))
            colsum_sb = small.tile([128, RB], F32)
            nc.vector.memset(colsum_sb, 0.0)

            imgT_sb = big.tile([D, B], F32)
            s_bc = big.tile([D, B], F32)
            txtTs_sb = small.tile([D, S], F32)
            txtTs_bf = small.tile([D, S], BF16)
            txtn_sb = small.tile([128, RB, D], F32)
            imgTs_sb = small.tile([D, S], F32)

            nc.sync.dma_start(txtTs_sb, txtTs[:, :])
            nc.sync.dma_start(imgTs_sb, imgTs[:, :])
            nc.sync.dma_start(
                txtn_sb, txtn[:, :].rearrange("(t p) d -> p t d", p=128)
            )
            nc.vector.tensor_copy(txtTs_bf, txtTs_sb)

            # ---------------- txt-side scales b = 1/(T*|y_j|) -------------
            # natural layout [p, t]: j = t*128 + p; b = exp(-ln(ss)/2 + ln(1/T))
            sq_t = small.tile([128, RB, D], F32)
            nc.vector.tensor_mul(sq_t, txtn_sb, txtn_sb)
            b_nat = small.tile([128, RB], F32)
            nc.vector.reduce_sum(b_nat, sq_t, axis=AX.X)
            nc.scalar.activation(b_nat, b_nat, AF.Ln)
            nc.scalar.activation(b_nat, b_nat, AF.Exp, bias=lninvT, scale=-0.5)

            # b in row layout [1, S] for the diagonal (j = 128*t + p),
            # via a DRAM bounce (partition<->free transposes don't balance
            # as direct SBUF->SBUF DMAs)
            b_row = small.tile([1, S], F32)
            nc.sync.dma_start(
                b_dram[:].rearrange("(t p) -> p t", p=128), b_nat[:, :]
            )
            nc.sync.dma_start(b_row, b_dram[:].rearrange("(a n) -> a n", a=1))

            # ---------------- img-side scales s_i = 1/|x_i| ---------------
            # Pipelined in NG rounds so the main loop can start after round 0.
            # s stays in row layout throughout: Ln reads the ones-matmul PSUM
            # row directly, Exp writes an SBUF row, one DMA parks it in DRAM
            # and a stride-0-partition DMA broadcasts it to [D, GW].  (The
            # earlier partition-transpose bounce cost 4 DRAM round-trips per
            # round and made setup, not the main loop, half the span.)
            for r in range(NG):
                csl = slice(r * GW, (r + 1) * GW)
                nc.sync.dma_start(imgT_sb[:, csl], imgT[:, csl])
                sq_r = chunks.tile([D, GW], F32, tag="sq")
                nc.vector.tensor_mul(sq_r, imgT_sb[:, csl], imgT_sb[:, csl])
                ss_ps = psumv.tile([1, GW], F32, tag="vec")
                for k in range(GW // 512):
                    nc.tensor.matmul(
                        ss_ps[0:1, k * 512 : (k + 1) * 512],
                        ones64,
                        sq_r[:, k * 512 : (k + 1) * 512],
                        start=True,
                        stop=True,
                    )
                s_row = chunks.tile([1, GW], F32, tag="srow")
                nc.scalar.activation(s_row, ss_ps, AF.Ln)
                nc.scalar.activation(s_row, s_row, AF.Exp, scale=-0.5)
                nc.sync.dma_start(
                    s_dram[csl].rearrange("(a n) -> a n", a=1), s_row
                )
                sd = s_dram[:].rearrange("(a n) -> a n", a=1)[0:1, csl]
                nc.gpsimd.dma_start(
                    s_bc[:, csl],
                    bass.AP(
                        tensor=sd.tensor,
                        offset=sd.offset,
                        ap=[[0, D]] + list(sd.ap[1:]),
                    ),
                )

            # ---------------- diagonal logits ------------------------------
            # d_j = (x_j . y_j) * (1/|x_j|) * (1/(T |y_j|))
            dmul = chunks.tile([D, S], F32, tag="dmul")
            nc.vector.tensor_mul(dmul, imgTs_sb, txtTs_sb)
            dot_ps = psumv.tile([1, S], F32, tag="vec")
            for k in range(S // 512):
                nc.tensor.matmul(
                    dot_ps[0:1, k * 512 : (k + 1) * 512],
                    ones64,
                    dmul[:, k * 512 : (k + 1) * 512],
                    start=True,
                    stop=True,
                )
            dot_sb = small.tile([1, S], F32)
            nc.vector.tensor_copy(dot_sb, dot_ps)
            sqis = chunks.tile([D, S], F32, tag="dmul")
            nc.vector.tensor_mul(sqis, imgTs_sb, imgTs_sb)
            ssis_ps = psumv.tile([1, S], F32, tag="vec")
            for k in range(S // 512):
                nc.tensor.matmul(
                    ssis_ps[0:1, k * 512 : (k + 1) * 512],
                    ones64,
                    sqis[:, k * 512 : (k + 1) * 512],
                    start=True,
                    stop=True,
                )
            s_is = small.tile([1, S], F32)
            nc.scalar.activation(s_is, ssis_ps, AF.Ln)
            nc.scalar.activation(s_is, s_is, AF.Exp, scale=-0.5)
            d_sb = small.tile([1, S], F32)
            nc.vector.tensor_mul(d_sb, dot_sb, s_is)
            nc.vector.tensor_mul(d_sb, d_sb, b_row)
            nc.sync.dma_start(diag[0:1, :], d_sb)

            # ---------------- main loop ------------------------------------
            # Per (g, rb): bf16 matmuls -> psum, exp(scale*psum - C) -> E
            # (accum_out = t2i partials). The 8 E tiles of a group are summed
            # elementwise on DVE+Pool (pairwise tree, overlapped with the
            # matmul stream), so the PE partition-reduce for the i2t partials
            # is ONE ones-matmul pass per group instead of one per E tile —
            # that reduce used to cost as many PE columns as the matmuls
            # themselves.  The final add emits bf16 so the ones-matmul stays
            # single-pass; the rounding is zero-mean and vanishes in the mean.
            NK = GW // 512

            def ones_pass(rs, accb, k):
                nc.tensor.matmul(
                    rs[0:1, k * 512 : (k + 1) * 512],
                    ones128,
                    accb[:, k * 512 : (k + 1) * 512],
                    start=True,
                    stop=True,
                )

            def flush_rowp(rs, g_idx):
                rowp_sb = chunks.tile([1, GW], F32, tag="rowp")
                nc.vector.tensor_copy(rowp_sb, rs)
                nc.sync.dma_start(
                    rowp[0:1, g_idx * GW : (g_idx + 1) * GW], rowp_sb
                )

            prev_accb = None
            for g in range(NG):
                gsl = slice(g * GW, (g + 1) * GW)
                imgTn_g = chunks.tile([D, GW], BF16, tag="imgTn")
                nc.vector.tensor_mul(imgTn_g, imgT_sb[:, gsl], s_bc[:, gsl])
                cur_e = []
                for rb in range(RB):
                    psum_t = psum.tile([128, GW], F32, tag="mm")
                    lhsT = txtTs_bf[:, rb * 128 : (rb + 1) * 128]
                    for k in range(NK):
                        nc.tensor.matmul(
                            psum_t[:, k * 512 : (k + 1) * 512],
                            lhsT,
                            imgTn_g[:, k * 512 : (k + 1) * 512],
                            start=True,
                            stop=True,
                        )
                        # prev group's reduced tile is ready by now; its two
                        # ones-passes slot between main matmuls (alternating
                        # weight sets keeps LDWEIGHTS in the background).
                        if prev_accb is not None and rb == 6:
                            if k == 0:
                                rs_ps = psumv.tile([1, GW], F32, tag="vec")
                            ones_pass(rs_ps, prev_accb, k)
                    e_t = etile.tile([128, GW], BF16, tag="e")
                    acc = accp.tile([128, 1], F32, tag="acc")
                    nc.scalar.activation(
                        e_t,
                        psum_t,
                        AF.Exp,
                        bias=negC,
                        scale=b_nat[:, rb : rb + 1],
                        accum_out=acc,
                    )
                    nc.gpsimd.tensor_add(
                        colsum_sb[:, rb : rb + 1],
                        colsum_sb[:, rb : rb + 1],
                        acc,
                    )
                    cur_e.append(e_t)
                    if rb == 6 and prev_accb is not None:
                        flush_rowp(rs_ps, g - 1)
                    # eager pairwise E reduction, all on DVE (Pool is ~2x
                    # slower per element and was lengthening the accb tail);
                    # running sum kept in t01 so the post-exp7 tail is just
                    # two adds before the next group's rb6 ones-pass.
                    if rb == 1:
                        t01 = tmpp.tile([128, GW], F32, tag="t01")
                        nc.vector.tensor_add(t01, cur_e[0], cur_e[1])
                    elif rb == 3:
                        t23 = tmpp.tile([128, GW], F32, tag="t23")
                        nc.vector.tensor_add(t23, cur_e[2], cur_e[3])
                        nc.vector.tensor_add(t01, t01, t23)
                    elif rb == 5:
                        t45 = tmpp.tile([128, GW], F32, tag="t45")
                        nc.vector.tensor_add(t45, cur_e[4], cur_e[5])
                        nc.vector.tensor_add(t01, t01, t45)
                    elif rb == 7:
                        t67 = tmpp.tile([128, GW], F32, tag="t67")
                        nc.vector.tensor_add(t67, cur_e[6], cur_e[7])
                        accb = tmpp.tile([128, GW], BF16, tag="accb")
                        nc.vector.tensor_add(accb, t01, t67)
                prev_accb = accb
            rs_ps = psumv.tile([1, GW], F32, tag="vec")
            for k in range(NK):
                ones_pass(rs_ps, prev_accb, k)
            flush_rowp(rs_ps, NG - 1)

            nc.sync.dma_start(colsum[:, :], colsum_sb)

    nc.compile()
    _patch_act_table_loads(nc)
    return nc


def _patch_act_table_loads(nc):
    """All ACT funcs used here (Ln, Exp) live in one table set
    ('natural_log_exp_and_others'), but the insertion pass picks a
    different set per function, reloading tables (~1.3us each) on every
    Ln<->Exp transition. Point every load at the combined set and drop
    the now-redundant reloads (keeping any that carry semaphore ops)."""
    from concourse.hw_specs import get_activation_tables

    tables = list(get_activation_tables(nc.m.arch).items())
    want = {"ln", "exp"}
    idx = next(
        i
        for i, (name, funcs) in enumerate(tables)
        if name == "natural_log_exp_and_others"
    )
    assert want <= {f.name.lower() for f in tables[idx][1]} or True
    seen = False
    for blk in nc.main_func.blocks:
        drop = []
        for ins in blk.instructions:
            if not isinstance(ins, mybir.InstLoadActFuncSet):
                continue
            ins.act_func_set_id = idx
            si = getattr(ins, "sync_info", None)
            clean = si is None or (not si.on_wait and not si.on_update)
            if seen and clean:
                drop.append(ins)
            seen = True
        for ins in drop:
            blk.instructions.remove(ins)


_NC = None


def _get_nc():
    global _NC
    if _NC is None:
        _NC = build_nc()
    return _NC


def kernel(image_features, text_features):
    img = np.ascontiguousarray(np.asarray(image_features, dtype=np.float32))
    txt = np.ascontiguousarray(np.asarray(text_features, dtype=np.float32))
    assert img.shape == (B, D) and txt.shape == (B, D)

    imgT = np.ascontiguousarray(img.T)
    in_maps = []
    for c in range(NCORES):
        sh = slice(c * S, (c + 1) * S)
        in_maps.append(
            {
                "imgT": imgT,
                "imgTs": np.ascontiguousarray(img[sh].T),
                "txtTs": np.ascontiguousarray(txt[sh].T),
                "txtn": np.ascontiguousarray(txt[sh]),
            }
        )

    trace = _maybe_enable_trace()
    res = run_bass_kernel_spmd(
        _get_nc(), in_maps, core_ids=list(range(NCORES)), trace=trace
    )
    global LAST_EXEC_TIME_NS
    LAST_EXEC_TIME_NS = res.exec_time_ns

    rowsum = np.zeros(B, dtype=np.float64)
    colsum = np.empty(B, dtype=np.float64)
    diag = np.empty(B, dtype=np.float64)
    for c in range(NCORES):
        r = res.results[c]
        rowsum += r["rowp"][0].astype(np.float64)
        colsum[c * S : (c + 1) * S] = r["colsum"].T.reshape(-1)
        diag[c * S : (c + 1) * S] = r["diag"][0]

    # exp was computed with bias -CMAX, so logZ = CMAX + log(sum)
    dlr = diag - (np.log(rowsum) + CMAX)  # diag of row log-softmax (i2t)
    dlc = diag - (np.log(colsum) + CMAX)  # diag of col log-softmax (t2i)
    p_pos = np.exp(dlr)
    w = (1.0 - p_pos) ** 2
    loss = 0.5 * (np.mean(w * -dlr) + np.mean(w * -dlc))
    return np.array(loss, dtype=np.float32)


# revision 30
# speedup vs baseline: 1.4983x; 1.0443x over previous
"""Distributed CLIP-style focal contrastive loss on 8 Trainium2 NeuronCores.

Math (reference): normalize img/txt rows, logits = img_n @ txt_n.T / T,
row/col log-softmax diagonals, focal-weighted mean.

Sharding: each core owns a 1024-row TEXT shard. Per core the logits block is
computed transposed: psum[p=txt_j, n=img_i] = txt_raw_j . (img_i / |img_i|),
with the txt norm and 1/T folded into the exp's per-partition scale, and a
constant bias of -1/T making exp(arg) <= 1 (stable softmax without a max
pass). The ScalarE exp's accum_out gives the complete t2i denominators for
the shard; a ones-matmul on PE (partition reduction) accumulated over row
blocks in PSUM gives partial i2t denominators, reduced across cores on the
host. Diagonal logits are computed separately from the shard vectors.
The final O(B) log/focal/mean is done on the host in float64.

Matmul operands are cast to bf16 (single-pass PE; fp32 matmuls lower to two
HW passes). The resulting per-logit noise (~6e-3 absolute) is zero-mean and
averages out over the 8192-term loss mean (~1e-4 relative on the loss).
Norms use exp(-0.5*ln(ss)) on ScalarE: same ACT table set as the main exp,
and avoids DVE reciprocal (iterative divide, ~5 ns/elem).
"""

import math
import os
import sys
import types

sys.path.insert(0, "/opt/trn_rl_repo")

import numpy as np

import concourse.bacc as bacc
import concourse.bass as bass
import concourse.tile as tile
from concourse import mybir
from concourse.bass_utils import run_bass_kernel_spmd

B = 8192
D = 64
NCORES = 8
S = B // NCORES  # 1024 txt rows per core
RB = S // 128  # 8 row blocks per shard
NG = 8  # img column groups
GW = B // NG  # 1024 img columns per group
TEMP = 0.07
CMAX = 1.0 / TEMP  # upper bound on any logit; used as the exp bias
F32 = mybir.dt.float32
BF16 = mybir.dt.bfloat16
AX = mybir.AxisListType
AF = mybir.ActivationFunctionType

LAST_EXEC_TIME_NS = None


def _maybe_enable_trace():
    """Wire up the NTFF profile hook (missing from this image's antenv) so
    run_bass_kernel_spmd(trace=True) can fetch real HW profiles via axon."""
    if not os.environ.get("BASS_KERNEL_TRACE"):
        return False
    try:
        if "antenv.axon_hooks" not in sys.modules:
            import trn_agent_boot.trn_boot as tb

            hook = tb._ntff_profile_via_ctypes("/opt/axon/libaxon_pjrt.so")
            mod = types.ModuleType("antenv.axon_hooks")
            mod.get_axon_ntff_profile_hook = lambda: hook
            sys.modules["antenv.axon_hooks"] = mod
        import concourse.bass_utils as bu

        bu.upload_artifacts = lambda tmpdir: "local://" + tmpdir
        return True
    except Exception:
        return False


def build_nc():
    nc = bacc.Bacc(None, target_bir_lowering=False)

    imgT = nc.dram_tensor("imgT", [D, B], F32, kind="ExternalInput")
    imgTs = nc.dram_tensor("imgTs", [D, S], F32, kind="ExternalInput")
    txtTs = nc.dram_tensor("txtTs", [D, S], F32, kind="ExternalInput")
    txtn = nc.dram_tensor("txtn", [S, D], F32, kind="ExternalInput")

    rowp = nc.dram_tensor("rowp", [1, B], F32, kind="ExternalOutput")
    colsum = nc.dram_tensor("colsum", [128, RB], F32, kind="ExternalOutput")
    diag = nc.dram_tensor("diag", [1, S], F32, kind="ExternalOutput")

    ss_dram = nc.dram_tensor("ss_scratch", [B], F32, kind="Internal")
    s_dram = nc.dram_tensor("s_scratch", [B], F32, kind="Internal")
    b_dram = nc.dram_tensor("b_scratch", [S], F32, kind="Internal")

    with tile.TileContext(nc) as tc:
        with (
            tc.tile_pool(name="big", bufs=1) as big,
            tc.tile_pool(name="chunks", bufs=2) as chunks,
            tc.tile_pool(name="small", bufs=1) as small,
            tc.tile_pool(name="etile", bufs=RB + 2) as etile,
            tc.tile_pool(name="acc", bufs=4) as accp,
            tc.tile_pool(name="tmp", bufs=2) as tmpp,
            tc.tile_pool(name="psum_mm", bufs=3, space="PSUM") as psum,
            tc.tile_pool(name="psum_vec", bufs=1, space="PSUM") as psumv,
        ):
            # ---------------- constants / persistent tiles ----------------
            ones64 = small.tile([D, 1], F32)
            nc.vector.memset(ones64, 1.0)
            ones128 = small.tile([128, 1], BF16)
            nc.vector.memset(ones128, 1.0)
            negC = small.tile([128, 1], F32)
            nc.vector.memset(negC, -CMAX)
            lninvT = small.tile([128, 1], F32)
            nc.vector.memset(lninvT, math.log(1.0 / TEMP))
            colsum_sb = small.tile([128, RB], F32)
            nc.vector.memset(colsum_sb, 0.0)

            imgT_sb = big.tile([D, B], F32)
            s_bc = big.tile([D, B], F32)
            txtTs_sb = small.tile([D, S], F32)
            txtTs_bf = small.tile([D, S], BF16)
            txtn_sb = small.tile([128, RB, D], F32)
            imgTs_sb = small.tile([D, S], F32)

            # Input loads: img group 0 and the main-matmul weights first on
            # the sync queue (they gate the first main-loop group); the txtn
            # gather (many small descriptors) rides the gpsimd queue so it
            # can't delay them.
            nc.sync.dma_start(imgT_sb[:, 0:GW], imgT[:, 0:GW])
            nc.sync.dma_start(txtTs_sb, txtTs[:, :])
            nc.gpsimd.dma_start(
                txtn_sb, txtn[:, :].rearrange("(t p) d -> p t d", p=128)
            )
            for r in range(1, NG):
                csl = slice(r * GW, (r + 1) * GW)
                nc.sync.dma_start(imgT_sb[:, csl], imgT[:, csl])
            nc.sync.dma_start(imgTs_sb, imgTs[:, :])
            nc.vector.tensor_copy(txtTs_bf, txtTs_sb)

            # ---------------- txt-side scales b = 1/(T*|y_j|) -------------
            # natural layout [p, t]: j = t*128 + p; b = exp(-ln(ss)/2 + ln(1/T))
            sq_t = small.tile([128, RB, D], F32)
            nc.vector.tensor_mul(sq_t, txtn_sb, txtn_sb)
            b_nat = small.tile([128, RB], F32)
            nc.vector.reduce_sum(b_nat, sq_t, axis=AX.X)
            nc.scalar.activation(b_nat, b_nat, AF.Ln)
            nc.scalar.activation(b_nat, b_nat, AF.Exp, bias=lninvT, scale=-0.5)

            # b in row layout [1, S] for the diagonal (j = 128*t + p),
            # via a DRAM bounce (partition<->free transposes don't balance
            # as direct SBUF->SBUF DMAs)
            b_row = small.tile([1, S], F32)
            nc.sync.dma_start(
                b_dram[:].rearrange("(t p) -> p t", p=128), b_nat[:, :]
            )
            nc.sync.dma_start(b_row, b_dram[:].rearrange("(a n) -> a n", a=1))

            # ---------------- img-side scales s_i = 1/|x_i| ---------------
            # Pipelined in NG rounds so the main loop can start after round 0.
            # s stays in row layout throughout: Ln reads the ones-matmul PSUM
            # row directly, Exp writes an SBUF row, one DMA parks it in DRAM
            # and a stride-0-partition DMA broadcasts it to [D, GW].  (The
            # earlier partition-transpose bounce cost 4 DRAM round-trips per
            # round and made setup, not the main loop, half the span.)
            for r in range(NG):
                csl = slice(r * GW, (r + 1) * GW)
                sq_r = chunks.tile([D, GW], F32, tag="sq")
                nc.vector.tensor_mul(sq_r, imgT_sb[:, csl], imgT_sb[:, csl])
                ss_ps = psumv.tile([1, GW], F32, tag="vec")
                for k in range(GW // 512):
                    nc.tensor.matmul(
                        ss_ps[0:1, k * 512 : (k + 1) * 512],
                        ones64,
                        sq_r[:, k * 512 : (k + 1) * 512],
                        start=True,
                        stop=True,
                    )
                s_row = chunks.tile([1, GW], F32, tag="srow")
                nc.scalar.activation(s_row, ss_ps, AF.Ln)
                nc.scalar.activation(s_row, s_row, AF.Exp, scale=-0.5)
                nc.sync.dma_start(
                    s_dram[csl].rearrange("(a n) -> a n", a=1), s_row
                )
                sd = s_dram[:].rearrange("(a n) -> a n", a=1)[0:1, csl]
                nc.gpsimd.dma_start(
                    s_bc[:, csl],
                    bass.AP(
                        tensor=sd.tensor,
                        offset=sd.offset,
                        ap=[[0, D]] + list(sd.ap[1:]),
                    ),
                )

            # ---------------- diagonal logits ------------------------------
            # d_j = (x_j . y_j) * (1/|x_j|) * (1/(T |y_j|))
            dmul = chunks.tile([D, S], F32, tag="dmul")
            nc.vector.tensor_mul(dmul, imgTs_sb, txtTs_sb)
            dot_ps = psumv.tile([1, S], F32, tag="vec")
            for k in range(S // 512):
                nc.tensor.matmul(
                    dot_ps[0:1, k * 512 : (k + 1) * 512],
                    ones64,
                    dmul[:, k * 512 : (k + 1) * 512],
                    start=True,
                    stop=True,
                )
            dot_sb = small.tile([1, S], F32)
            nc.vector.tensor_copy(dot_sb, dot_ps)
            sqis = chunks.tile([D, S], F32, tag="dmul")
            nc.vector.tensor_mul(sqis, imgTs_sb, imgTs_sb)
            ssis_ps = psumv.tile([1, S], F32, tag="vec")
            for k in range(S // 512):
                nc.tensor.matmul(
                    ssis_ps[0:1, k * 512 : (k + 1) * 512],
                    ones64,
                    sqis[:, k * 512 : (k + 1) * 512],
                    start=True,
                    stop=True,
                )
            s_is = small.tile([1, S], F32)
            nc.scalar.activation(s_is, ssis_ps, AF.Ln)
            nc.scalar.activation(s_is, s_is, AF.Exp, scale=-0.5)
            d_sb = small.tile([1, S], F32)
            nc.vector.tensor_mul(d_sb, dot_sb, s_is)
            nc.vector.tensor_mul(d_sb, d_sb, b_row)
            nc.sync.dma_start(diag[0:1, :], d_sb)

            # ---------------- main loop ------------------------------------
            # Per (g, rb): bf16 matmuls -> psum, exp(scale*psum - C) -> E
            # (accum_out = t2i partials). The 8 E tiles of a group are summed
            # elementwise on DVE+Pool (pairwise tree, overlapped with the
            # matmul stream), so the PE partition-reduce for the i2t partials
            # is ONE ones-matmul pass per group instead of one per E tile —
            # that reduce used to cost as many PE columns as the matmuls
            # themselves.  The final add emits bf16 so the ones-matmul stays
            # single-pass; the rounding is zero-mean and vanishes in the mean.
            NK = GW // 512

            def ones_pass(rs, accb, k):
                nc.tensor.matmul(
                    rs[0:1, k * 512 : (k + 1) * 512],
                    ones128,
                    accb[:, k * 512 : (k + 1) * 512],
                    start=True,
                    stop=True,
                )

            def flush_rowp(rs, g_idx):
                rowp_sb = chunks.tile([1, GW], F32, tag="rowp")
                nc.vector.tensor_copy(rowp_sb, rs)
                nc.sync.dma_start(
                    rowp[0:1, g_idx * GW : (g_idx + 1) * GW], rowp_sb
                )

            def make_imgTn(g_idx):
                gsl = slice(g_idx * GW, (g_idx + 1) * GW)
                t = chunks.tile([D, GW], BF16, tag="imgTn")
                nc.vector.tensor_mul(t, imgT_sb[:, gsl], s_bc[:, gsl])
                return t

            prev_accb = None
            imgTn_g = make_imgTn(0)
            for g in range(NG):
                cur_e = []
                for rb in range(RB):
                    psum_t = psum.tile([128, GW], F32, tag="mm")
                    lhsT = txtTs_bf[:, rb * 128 : (rb + 1) * 128]
                    for k in range(NK):
                        nc.tensor.matmul(
                            psum_t[:, k * 512 : (k + 1) * 512],
                            lhsT,
                            imgTn_g[:, k * 512 : (k + 1) * 512],
                            start=True,
                            stop=True,
                        )
                        # prev group's reduced tile is ready by now; its two
                        # ones-passes slot between main matmuls (alternating
                        # weight sets keeps LDWEIGHTS in the background).
                        if prev_accb is not None and rb == 6:
                            if k == 0:
                                rs_ps = psumv.tile([1, GW], F32, tag="vec")
                            ones_pass(rs_ps, prev_accb, k)
                    e_t = etile.tile([128, GW], BF16, tag="e")
                    acc = accp.tile([128, 1], F32, tag="acc")
                    nc.scalar.activation(
                        e_t,
                        psum_t,
                        AF.Exp,
                        bias=negC,
                        scale=b_nat[:, rb : rb + 1],
                        accum_out=acc,
                    )
                    nc.gpsimd.tensor_add(
                        colsum_sb[:, rb : rb + 1],
                        colsum_sb[:, rb : rb + 1],
                        acc,
                    )
                    cur_e.append(e_t)
                    if rb == 2 and g + 1 < NG:
                        # next group's scaled-img tile, emitted here so it
                        # sits ahead of this group's tail adds in the DVE
                        # queue — otherwise the next group's first matmul
                        # serializes on this group's accb chain.
                        imgTn_next = make_imgTn(g + 1)
                    if rb == 6 and prev_accb is not None:
                        flush_rowp(rs_ps, g - 1)
                    # eager pairwise E reduction: leaf pairs t23/t45 go to
                    # Pool (2x slower per element but off the critical path),
                    # the running sum t01 and the post-exp7 tail stay on DVE
                    # so accb lands well before the next group's rb6
                    # ones-pass.
                    if rb == 1:
                        t01 = tmpp.tile([128, GW], F32, tag="t01")
                        nc.vector.tensor_add(t01, cur_e[0], cur_e[1])
                    elif rb == 3:
                        t23 = tmpp.tile([128, GW], F32, tag="t23")
                        nc.gpsimd.tensor_add(t23, cur_e[2], cur_e[3])
                        nc.vector.tensor_add(t01, t01, t23)
                    elif rb == 5:
                        t45 = tmpp.tile([128, GW], F32, tag="t45")
                        nc.gpsimd.tensor_add(t45, cur_e[4], cur_e[5])
                        nc.vector.tensor_add(t01, t01, t45)
                    elif rb == 7:
                        t67 = tmpp.tile([128, GW], F32, tag="t67")
                        nc.vector.tensor_add(t67, cur_e[6], cur_e[7])
                        accb = tmpp.tile([128, GW], BF16, tag="accb")
                        nc.vector.tensor_add(accb, t01, t67)
                prev_accb = accb
                if g + 1 < NG:
                    imgTn_g = imgTn_next
            rs_ps = psumv.tile([1, GW], F32, tag="vec")
            for k in range(NK):
                ones_pass(rs_ps, prev_accb, k)
            flush_rowp(rs_ps, NG - 1)

            nc.sync.dma_start(colsum[:, :], colsum_sb)

    nc.compile()
    _patch_act_table_loads(nc)
    return nc


def _patch_act_table_loads(nc):
    """All ACT funcs used here (Ln, Exp) live in one table set
    ('natural_log_exp_and_others'), but the insertion pass picks a
    different set per function, reloading tables (~1.3us each) on every
    Ln<->Exp transition. Point every load at the combined set and drop
    the now-redundant reloads (keeping any that carry semaphore ops)."""
    from concourse.hw_specs import get_activation_tables

    tables = list(get_activation_tables(nc.m.arch).items())
    want = {"ln", "exp"}
    idx = next(
        i
        for i, (name, funcs) in enumerate(tables)
        if name == "natural_log_exp_and_others"
    )
    assert want <= {f.name.lower() for f in tables[idx][1]} or True
    seen = False
    for blk in nc.main_func.blocks:
        drop = []
        for ins in blk.instructions:
            if not isinstance(ins, mybir.InstLoadActFuncSet):
                continue
            ins.act_func_set_id = idx
            si = getattr(ins, "sync_info", None)
            clean = si is None or (not si.on_wait and not si.on_update)
            if seen and clean:
                drop.append(ins)
            seen = True
        for ins in drop:
            blk.instructions.remove(ins)


_NC = None


def _get_nc():
    global _NC
    if _NC is None:
        _NC = build_nc()
    return _NC


def kernel(image_features, text_features):
    img = np.ascontiguousarray(np.asarray(image_features, dtype=np.float32))
    txt = np.ascontiguousarray(np.asarray(text_features, dtype=np.float32))
    assert img.shape == (B, D) and txt.shape == (B, D)

    imgT = np.ascontiguousarray(img.T)
    in_maps = []
    for c in range(NCORES):
        sh = slice(c * S, (c + 1) * S)
        in_maps.append(
            {
                "imgT": imgT,
                "imgTs": np.ascontiguousarray(img[sh].T),
                "txtTs": np.ascontiguousarray(txt[sh].T),
                "txtn": np.ascontiguousarray(txt[sh]),
            }
        )

    trace = _maybe_enable_trace()
    res = run_bass_kernel_spmd(
        _get_nc(), in_maps, core_ids=list(range(NCORES)), trace=trace
    )
    global LAST_EXEC_TIME_NS
    LAST_EXEC_TIME_NS = res.exec_time_ns

    rowsum = np.zeros(B, dtype=np.float64)
    colsum = np.empty(B, dtype=np.float64)
    diag = np.empty(B, dtype=np.float64)
    for c in range(NCORES):
        r = res.results[c]
        rowsum += r["rowp"][0].astype(np.float64)
        colsum[c * S : (c + 1) * S] = r["colsum"].T.reshape(-1)
        diag[c * S : (c + 1) * S] = r["diag"][0]

    # exp was computed with bias -CMAX, so logZ = CMAX + log(sum)
    dlr = diag - (np.log(rowsum) + CMAX)  # diag of row log-softmax (i2t)
    dlc = diag - (np.log(colsum) + CMAX)  # diag of col log-softmax (t2i)
    p_pos = np.exp(dlr)
    w = (1.0 - p_pos) ** 2
    loss = 0.5 * (np.mean(w * -dlr) + np.mean(w * -dlc))
    return np.array(loss, dtype=np.float32)
